# revision 42
# baseline (speedup 1.0000x reference)
"""Multi-head attention (B=4, N=1024, C=1024, H=16, D=64) on 8 Trainium2 cores.

Sharding: batch x head-half tensor parallel, no collectives. Core i handles
batch b = i//2 and heads (i%2)*8..+8 for ALL 1024 queries of that batch: it
projects q/k/v for its 8 heads only (no duplicated k/v work between the two
cores of a batch), runs attention, and computes the PARTIAL output projection
y_i = o_i @ w_out[rows of its 512 e-dims]. The host sums each batch's two
partials and adds the bias -- the output projection is linear in the head
dimension, so the pair-sum equals the full projection.

Matmuls run in fp16 (1 PE column/cycle, weight loads hidden under streams).
Accumulation is fp32 in PSUM. exp is computed as exp(s/8 - 12*ln2) so
unnormalized attention outputs stay in fp16 range; the 2^-12 factor cancels
in the softmax normalization. The softmax denominator rides along as a
ones-column in v (key mask folded into both); v tiles are padded to 128
weight columns (65..127 zero) so AV matmuls get FWL.

v2 schedule changes vs v1:
  - Input DMA is 7 large descriptors (x in 4 quarters split across the two
    hwdge queues, wk0/wq0/mask on sync) instead of 33 small ones: a single
    dma_start is striped across all 16 SDMA engines, so descriptor-gen
    serialization (~0.6us each) was the real startup cost.
  - Normalization is per head-PAIR: the denominator reciprocal is computed
    by DVE directly from the AV PSUM row (no den staging copy), the two
    reciprocal rows of a pair live at partitions 64r/64r+1 and a single
    [2,128]-stationary PE matmul broadcasts both to the pair's 128 output
    partitions, and one [128,512] DVE multiply normalizes the whole pair.
    The broadcast PSUM tile borrows the ps_proj ring so the pav ring never
    deadlocks, and part2 (cast+broadcast+mul) is emitted one matmul group
    late so the PE queue never waits on the DVE reciprocal chain.
  - The output projection is interleaved with the last pair's norm:
    n-tiles 0..3 (query half 0) run between norm(3,qh0) and norm(3,qh1),
    n-tiles 4..7 after.

Per-core pipeline (x^T and packed weights are prepared on the host):
  1. Per head pair t (4 pairs): k^T/q^T column projections (K=co tiles),
     then S^T = k^T.T @ q^T per key m-tile, the pair alternating PE row
     groups 0/64 so its two matmuls overlap -> exp on ACT -> E.
  2. v = x @ w_v -> [m, 8 heads, d + ones column], mask folded in.
  3. Per pair and q-half: out^T (unnormalized) + denominator via the ones
     column -> o_un; fast reciprocal from PSUM, pair broadcast, normalize.
  4. y_partial = o^T.T @ w_out (K=4 e-tiles), fp16, DMA out.
"""

import os

import numpy as np

import concourse.bacc as bacc
import concourse.mybir as mybir
import concourse.tile as tile
from concourse.bass_utils import run_bass_kernel_spmd

F32 = mybir.dt.float32
F32R = mybir.dt.float32r
F16 = mybir.dt.float16

B, N, C = 4, 1024, 1024
H, D = 16, 64
P = 128
CO = C // P       # 8 contraction tiles
MO = N // P       # 8 key m-tiles
NO = N // P       # 8 output row tiles
HL = 8            # heads per core
T = HL // 2       # 4 head pairs per core
EO = T            # 4 e-tiles (one per pair) for the output projection
NH = N // 2       # 512-column matmul streams (PSUM bank)
ATT_SCALE = D ** -0.5
EXP_BIAS = float(-12.0 * np.log(2.0))  # keep out^T in fp16 range
N_CORES = 8


def build_nc():
    nc = bacc.Bacc()
    xbT = nc.declare_dram_parameter("xbT", [C, N], F16, isOutput=False)
    maskb = nc.declare_dram_parameter("maskb", [N], F32, isOutput=False)
    wq_pk = nc.declare_dram_parameter("wq_pk", [T, P, CO, P], F16,
                                      isOutput=False)
    wk_pk = nc.declare_dram_parameter("wk_pk", [T, P, CO, P], F16,
                                      isOutput=False)
    wv_pk = nc.declare_dram_parameter("wv_pk", [P, CO, HL * D], F16,
                                      isOutput=False)
    wo_pk = nc.declare_dram_parameter("wo_pk", [P, EO, C], F16, isOutput=False)
    ones2b = nc.declare_dram_parameter("ones2b", [P, P], F16, isOutput=False)
    y = nc.declare_dram_parameter("y", [N, C], F16, isOutput=True)
    debug = os.environ.get("KERNEL_DEBUG_DUMP") == "1"
    if debug:
        dbg_rcp = nc.declare_dram_parameter("dbg_rcp", [P, N], F32,
                                            isOutput=True)
        dbg_osb = nc.declare_dram_parameter("dbg_osb", [P, EO, N], F16,
                                            isOutput=True)
        dbg_kT = nc.declare_dram_parameter("dbg_kT", [P, T, N], F16,
                                           isOutput=True)
        dbg_vsb = nc.declare_dram_parameter("dbg_vsb", [P, MO, HL, P], F16,
                                            isOutput=True)

    xbT_t = xbT.rearrange("(co p) m -> p co m", p=P)
    y_t = y.rearrange("(no p) c -> p no c", p=P)

    with tile.TileContext(nc) as tc:
        with tc.tile_pool(name="consts", bufs=1) as consts, \
             tc.tile_pool(name="persist", bufs=1) as persist:
            # ---- constants ----
            # zero scratch for PE warm-up matmuls (first DVE op emitted so
            # the dummies can start as early as possible)
            scratch = consts.tile([P, NH], F16)
            nc.vector.memset(scratch[:], 0.0)
            onesH = consts.tile([P, HL], F16)
            nc.vector.memset(onesH[:], 1.0)
            # pair-broadcast stationary: row 64r+32j has ones in cols
            # 64j..64j+63, all other rows zero.  DMA'd from DRAM: engine
            # writes must be 32-partition aligned.
            ones2 = consts.tile([P, P], F16)
            mask_sb = consts.tile([P, MO], F32)
            ebias = consts.tile([P, 1], F32)
            nc.vector.memset(ebias[:], EXP_BIAS)

            # ---- persistent tensors ----
            qT = persist.tile([P, T, N], F16)            # q^T: [e, n]
            kT = persist.tile([P, T, N], F16)            # k^T: [e, m]
            v_sb = persist.tile([P, MO, HL, P], F16)     # v + ones col + pad
            o_sb = persist.tile([P, EO, N], F16)         # o^T: [e, n]
            wo = persist.tile([P, EO, C], F16)           # w_out staged late
            # den/rcp rows for head h=2t+j live at partition 64(t%2)+32j:
            # engine ops can only address partition bases 0/32/64/96, and
            # the custom-DVE reciprocal only works at base 0 (full tile).
            den_sb = persist.tile([97, N], F32)
            nc.vector.memset(den_sb[:], 1.0)             # keep rcp defined
            rcp32 = persist.tile([97, N], F32)
            rcp16 = persist.tile([P, N], F16)
            nc.vector.memset(rcp16[:], 0.0)              # unwritten rows: no
                                                         # NaN under the bcast

            with tc.tile_pool(name="xT_pool", bufs=1) as xT_pool, \
                 tc.tile_pool(name="w_pool", bufs=4) as w_pool, \
                 tc.tile_pool(name="wv_pool", bufs=1) as wv_pool, \
                 tc.tile_pool(name="E_pool", bufs=3) as E_pool, \
                 tc.tile_pool(name="ou_pool", bufs=2) as ou_pool, \
                 tc.tile_pool(name="ysb_pool", bufs=4) as ysb_pool, \
                 tc.tile_pool(name="ps_proj", bufs=2, space="PSUM") as ps_proj, \
                 tc.tile_pool(name="ps_s", bufs=2, space="PSUM") as ps_s, \
                 tc.tile_pool(name="ps_av", bufs=2, space="PSUM") as ps_av:
                xT = xT_pool.tile([P, CO, N], F16)       # x^T: [c, m]

                # ---- startup DMA: few LARGE descriptors (each dma_start is
                # striped across all 16 SDMA engines; it's the per-descriptor
                # generation cost ~0.6us that serializes on the queue).
                # sync: wk0, x(h0), wq0, mask; scalar: x(h1) pieces.
                wk0 = w_pool.tile([P, CO, P], F16, tag="wqk")
                wq0 = w_pool.tile([P, CO, P], F16, tag="wqk")
                nc.sync.dma_start(wk0[:], wk_pk[0])
                nc.scalar.dma_start(xT[:, 0:2, 0:NH], xbT_t[:, 0:2, 0:NH])
                nc.sync.dma_start(xT[:, 2:4, 0:NH], xbT_t[:, 2:4, 0:NH])
                nc.scalar.dma_start(xT[:, 4:6, 0:NH], xbT_t[:, 4:6, 0:NH])
                nc.sync.dma_start(xT[:, 6:CO, 0:NH], xbT_t[:, 6:CO, 0:NH])
                nc.sync.dma_start(wq0[:], wq_pk[0])
                nc.scalar.dma_start(xT[:, 0:4, NH:N], xbT_t[:, 0:4, NH:N])
                nc.sync.dma_start(xT[:, 4:CO, NH:N], xbT_t[:, 4:CO, NH:N])

                # PE warm-up: dummy matmuls on zero scratch while the first
                # x/w descriptors land, so the p-state ramp (first ~3us of
                # matmuls run ~2.8x slow) is paid before real work arrives
                for w in range(10):
                    pw = ps_av.tile([P, NH], F32, tag="ps_av",
                                    name=f"warm{w}")
                    nc.tensor.matmul(pw[:], scratch[:, 0:P], scratch[:],
                                     start=True, stop=True)

                def kq_half(t, half, wk, wq):
                    pk = ps_proj.tile([P, NH], F32, tag="pp",
                                      name=f"pk{t}_{half}")
                    for co in range(CO):
                        nc.tensor.matmul(
                            pk[:], wk[:, co, :],
                            xT[:, co, half * NH:(half + 1) * NH],
                            start=(co == 0), stop=(co == CO - 1))
                    nc.vector.tensor_copy(
                        kT[:, t, half * NH:(half + 1) * NH], pk[:])
                    pq = ps_proj.tile([P, NH], F32, tag="pp",
                                      name=f"pq{t}_{half}")
                    for co in range(CO):
                        nc.tensor.matmul(
                            pq[:], wq[:, co, :],
                            xT[:, co, half * NH:(half + 1) * NH],
                            start=(co == 0), stop=(co == CO - 1))
                    nc.vector.tensor_copy(
                        qT[:, t, half * NH:(half + 1) * NH], pq[:])

                def kq_proj(t, wk=None, wq=None):
                    if wk is None:
                        wk = w_pool.tile([P, CO, P], F16, tag="wqk",
                                         name=f"wk{t}")
                        nc.sync.dma_start(wk[:], wk_pk[t])
                    if wq is None:
                        wq = w_pool.tile([P, CO, P], F16, tag="wqk",
                                         name=f"wq{t}")
                        nc.sync.dma_start(wq[:], wq_pk[t])
                    kq_half(t, 0, wk, wq)
                    kq_half(t, 1, wk, wq)

                def s_exp_tiles(t, E_t, mo_list, qh_list):
                    # S^T + exp: one [128, 1024] psum per (mo, q-half) holds
                    # both heads of the pair (the back-to-back matmuls
                    # alternate PE row groups 0/64 and overlap on the array);
                    # one strided EXP covers both heads
                    for mo in mo_list:
                        for qh in qh_list:
                            pss = ps_s.tile([P, 2 * NH], F32, tag="ps_s",
                                            name=f"pss{t}_{mo}_{qh}")
                            for j in range(2):
                                pb = 64 * j
                                nc.tensor.matmul(
                                    pss[:, j * NH:(j + 1) * NH],
                                    kT[pb:pb + 64, t, mo * P:(mo + 1) * P],
                                    qT[pb:pb + 64, t, qh * NH:(qh + 1) * NH],
                                    start=True, stop=True)
                            nc.scalar.activation(
                                E_t[:, mo, :, qh * NH:(qh + 1) * NH],
                                pss[:].rearrange("p (j n) -> p j n", j=2),
                                mybir.ActivationFunctionType.Exp,
                                bias=ebias[:], scale=ATT_SCALE)

                def s_exp(t):
                    E_t = E_pool.tile([P, MO, 2, N], F16, tag="E",
                                      name=f"E{t}")
                    s_exp_tiles(t, E_t, range(MO), range(2))
                    return E_t

                def v_proj():
                    wv = wv_pool.tile([P, CO, HL * D], F16, tag="wv")
                    nc.sync.dma_start(wv[:], wv_pk[:])
                    for mo in range(MO):
                        pv = ps_proj.tile([P, NH], F32, tag="pp",
                                          name=f"pv{mo}")
                        for co in range(CO):
                            nc.tensor.matmul(
                                pv[:], xT[:, co, mo * P:(mo + 1) * P],
                                wv[:, co, :],
                                start=(co == 0), stop=(co == CO - 1))
                        nc.vector.tensor_scalar_mul(
                            v_sb[:, mo, :, 0:D],
                            pv[:].rearrange("p (h d) -> p h d", d=D),
                            mask_sb[:, mo:mo + 1])
                    for mo in range(MO):
                        nc.vector.tensor_scalar_mul(
                            v_sb[:, mo, :, D], onesH[:], mask_sb[:, mo:mo + 1])

                o_uns = {}

                def norm_part2(t, qh):
                    # cast the pair's two reciprocal rows (64r, 64r+32; the
                    # 31 rows between are zero so one 33-row cast is safe),
                    # broadcast them to the pair's 128 output partitions with
                    # one PE matmul (K=64 stationary, zeros except the two
                    # one-rows), then one [128,512] DVE multiply normalizes
                    # the whole pair.  Emitted >=1 matmul group after the
                    # reciprocals so the in-order PE queue never waits on the
                    # DVE chain.
                    r = t % 2
                    sl = slice(qh * NH, (qh + 1) * NH)
                    nc.vector.reciprocal_approx_fast(rcp32[:, sl],
                                                     den_sb[:, sl])
                    nc.vector.tensor_copy(rcp16[64 * r:64 * r + 33, sl],
                                          rcp32[64 * r:64 * r + 33, sl])
                    pbc = ps_proj.tile([P, NH], F32, tag="pp",
                                       name=f"pbc{t}_{qh}")
                    nc.tensor.matmul(
                        pbc[:], ones2[64 * r:64 * r + 64, :],
                        rcp16[64 * r:64 * r + 64, sl],
                        start=True, stop=True, tile_position=(64 * r, 0))
                    o_un = o_uns[t] if qh == 0 else o_uns.pop(t)
                    nc.vector.tensor_mul(o_sb[:, t, sl], o_un[:, sl], pbc[:])

                def av(t, E_t, mid=None):
                    # unnormalized out^T + denominator via the ones column;
                    # `mid` (if set) is emitted between the two query halves
                    # and takes over part2(t, 0) emission
                    r = t % 2
                    o_un = ou_pool.tile([P, N], F16, tag="ou", name=f"ou{t}")
                    o_uns[t] = o_un
                    for qh in range(2):
                        sl = slice(qh * NH, (qh + 1) * NH)
                        for j in range(2):
                            h = 2 * t + j
                            pav = ps_av.tile([P, NH], F32, tag="ps_av",
                                             name=f"pav{h}_{qh}")
                            for mo in range(MO):
                                nc.tensor.matmul(
                                    pav[:], v_sb[:, mo, h, :],
                                    E_t[:, mo, j, sl],
                                    start=(mo == 0), stop=(mo == MO - 1))
                            nc.vector.tensor_copy(
                                o_un[64 * j:64 * j + 64, sl], pav[0:D, :])
                            nc.vector.tensor_copy(
                                den_sb[64 * r + 32 * j:64 * r + 32 * j + 1,
                                       sl],
                                pav[D:D + 1, :])
                        if qh == 0 and mid is not None:
                            mid()
                        if qh == 1 and mid is None:
                            norm_part2(t, 0)

                def out_proj(no_list, last=False):
                    # one merged y descriptor per n-tile ([128, 1024]); the
                    # very last tile is split across both hwdge queues so the
                    # final transfer tail is halved
                    for no in no_list:
                        ysb = ysb_pool.tile([P, N], F16, tag="ysb",
                                            name=f"ysb{no}")
                        for ch in range(2):
                            pool = ps_proj if (no * 2 + ch) % 2 == 0 else ps_av
                            py = pool.tile([P, NH], F32,
                                           tag="pp" if pool is ps_proj
                                           else "ps_av",
                                           name=f"py{no}_{ch}")
                            for eo in range(EO):
                                nc.tensor.matmul(
                                    py[:], o_sb[:, eo, no * P:(no + 1) * P],
                                    wo[:, eo, ch * NH:(ch + 1) * NH],
                                    start=(eo == 0), stop=(eo == EO - 1))
                            if last and no == no_list[-1]:
                                if ch == 0:
                                    nc.scalar.copy(
                                        ysb[:, 0:NH], py[:])
                                    nc.sync.dma_start(
                                        y_t[:, no, 0:NH], ysb[:, 0:NH])
                                else:
                                    nc.vector.tensor_copy(
                                        ysb[:, NH:N], py[:])
                                    nc.sync.dma_start(
                                        y_t[:, no, NH:NH + 256],
                                        ysb[:, NH:NH + 256])
                                    nc.scalar.dma_start(
                                        y_t[:, no, NH + 256:N],
                                        ysb[:, NH + 256:N])
                            else:
                                # ch0 on ACT, ch1 on DVE so neither engine
                                # paces the out-proj matmul stream
                                if ch == 0:
                                    nc.scalar.copy(ysb[:, 0:NH], py[:])
                                else:
                                    nc.vector.tensor_copy(
                                        ysb[:, NH:N], py[:])
                        if not (last and no == no_list[-1]):
                            q = nc.sync if no % 2 == 0 else nc.scalar
                            q.dma_start(y_t[:, no, :], ysb[:])

                # skewed pipeline: attention of pair t overlaps k/q
                # projections of pair t+1 and S/exp of pair t+1.
                # pair-0 ramp: project the first query half, then run its
                # first S tiles + exps while x's second half is still landing
                kq_half(0, 0, wk0, wq0)
                E0 = E_pool.tile([P, MO, 2, N], F16, tag="E", name="E0")
                s_exp_tiles(0, E0, range(4), [0])
                kq_half(0, 1, wk0, wq0)
                s_exp_tiles(0, E0, range(4), [1])
                s_exp_tiles(0, E0, range(4, MO), range(2))
                Es = {0: E0}
                kq_proj(1)
                # non-critical input DMAs + the AV weight-pad memset go after
                # the pair-1 weight descriptors so they can't delay the ramp
                nc.sync.dma_start(mask_sb[:],
                                  maskb.rearrange("(o p) -> p o", p=P))
                nc.sync.dma_start(ones2[:], ones2b[:])
                nc.vector.memset(v_sb[:, :, :, D + 1:], 0.0)
                Es[1] = s_exp(1)
                kq_proj(2)
                v_proj()
                for t in range(T):
                    if t >= 1:
                        norm_part2(t - 1, 1)
                    if t + 2 < T:
                        E_n = E_pool.tile([P, MO, 2, N], F16, tag="E",
                                          name=f"E{t + 2}")
                        if t + 2 == T - 1:
                            # the last pair's exps run qh-major so its qh0
                            # AV/norm (and the first out-proj tiles) can
                            # overlap the qh1 exp stream
                            s_exp_tiles(t + 2, E_n, range(MO), [0])
                            s_exp_tiles(t + 2, E_n, range(MO), [1])
                        else:
                            s_exp_tiles(t + 2, E_n, range(MO), range(2))
                        Es[t + 2] = E_n
                    if t + 3 < T:
                        kq_proj(t + 3)
                    if t == 1:  # stage w_out late, off the critical DMA path
                        nc.sync.dma_start(wo[:], wo_pk[:])
                    if t == T - 1:
                        # last pair: norm qh0 + the first out-proj n-tiles
                        # run between the query halves, inside the qh1 exp
                        # window; the rest follows after qh1's norm
                        av(t, Es.pop(t),
                           mid=lambda: (norm_part2(3, 0),
                                        out_proj([0, 1, 2])))
                    else:
                        av(t, Es.pop(t))

                # ---- remaining output projection (host adds bias)
                out_proj([3])
                norm_part2(3, 1)
                out_proj([4, 5, 6, 7], last=True)

                if debug:
                    nc.sync.dma_start(dbg_rcp[0:97], rcp32[:])
                    nc.sync.dma_start(dbg_osb[:], o_sb[:])
                    nc.sync.dma_start(dbg_kT[:], kT[:])
                    nc.sync.dma_start(dbg_vsb[:], v_sb[:])

    nc.finalize()
    return nc


_NC_CACHE = None


def _get_nc():
    global _NC_CACHE
    if _NC_CACHE is None:
        _NC_CACHE = build_nc()
    return _NC_CACHE


def _make_in_maps(x, mask, w_qkv, w_out, b_out):
    x = np.asarray(x, dtype=np.float32)
    mask_f = np.asarray(mask).astype(np.float32)
    wqkv_h = np.asarray(w_qkv).astype(np.float16)
    wout_h = np.asarray(w_out).astype(np.float16)
    # w_qkv [C, 3HD]: q cols 0:C, k cols C:2C, v cols 2C:3C; head h at h*D
    wq4 = wqkv_h.reshape(CO, P, 3 * H * D)
    in_maps = []
    for i in range(N_CORES):
        b, hh = i // 2, i % 2
        e0 = hh * HL * D  # first e-col of this core's head block
        xbT = np.ascontiguousarray(x[b].T.astype(np.float16))
        wq_pk = np.ascontiguousarray(
            wq4[:, :, e0:e0 + HL * D].reshape(CO, P, T, P)
            .transpose(2, 1, 0, 3))
        wk_pk = np.ascontiguousarray(
            wq4[:, :, C + e0:C + e0 + HL * D].reshape(CO, P, T, P)
            .transpose(2, 1, 0, 3))
        wv_pk = np.ascontiguousarray(
            wq4[:, :, 2 * C + e0:2 * C + e0 + HL * D].transpose(1, 0, 2))
        wo_pk = np.ascontiguousarray(
            wout_h[e0:e0 + HL * D, :].reshape(EO, P, C).transpose(1, 0, 2))
        ones2b = np.zeros((P, P), np.float16)
        for r in range(2):
            ones2b[64 * r, 0:64] = 1.0
            ones2b[64 * r + 32, 64:128] = 1.0
        in_maps.append({"xbT": xbT, "maskb": mask_f[b], "wq_pk": wq_pk,
                        "wk_pk": wk_pk, "wv_pk": wv_pk, "wo_pk": wo_pk,
                        "ones2b": ones2b})
    return in_maps


def run_kernel(x, mask, w_qkv, w_out, b_out, trace=False):
    """Run on 8 cores; returns (full output [B,N,C], BassKernelResults)."""
    nc = _get_nc()
    in_maps = _make_in_maps(x, mask, w_qkv, w_out, b_out)
    res = run_bass_kernel_spmd(nc, in_maps, core_ids=list(range(N_CORES)),
                               trace=trace)
    bias = np.asarray(b_out, dtype=np.float32)
    out = np.empty((B, N, C), dtype=np.float32)
    for b in range(B):
        out[b] = (res.results[2 * b]["y"].astype(np.float32)
                  + res.results[2 * b + 1]["y"].astype(np.float32) + bias)
    return out, res


def kernel(x, mask, w_qkv, w_out, b_out):
    os.environ.setdefault("BASS_NEVER_TRACE", "1")
    out, _ = run_kernel(x, mask, w_qkv, w_out, b_out, trace=False)
    return out


# revision 43
# speedup vs baseline: 1.0150x; 1.0150x over previous
"""Multi-head attention (B=4, N=1024, C=1024, H=16, D=64) on 8 Trainium2 cores.

Sharding: batch x head-half tensor parallel, no collectives. Core i handles
batch b = i//2 and heads (i%2)*8..+8 for ALL 1024 queries of that batch: it
projects q/k/v for its 8 heads only (no duplicated k/v work between the two
cores of a batch), runs attention, and computes the PARTIAL output projection
y_i = o_i @ w_out[rows of its 512 e-dims]. The host sums each batch's two
partials and adds the bias -- the output projection is linear in the head
dimension, so the pair-sum equals the full projection.

Matmuls run in fp16 (1 PE column/cycle, weight loads hidden under streams).
Accumulation is fp32 in PSUM. exp is computed as exp(s/8 - 12*ln2) so
unnormalized attention outputs stay in fp16 range; the 2^-12 factor cancels
in the softmax normalization. The softmax denominator rides along as a
ones-column in v (key mask folded into both); v tiles are padded to 128
weight columns (65..127 zero) so AV matmuls get FWL.

v2 schedule changes vs v1:
  - Input DMA is 7 large descriptors (x in 4 quarters split across the two
    hwdge queues, wk0/wq0/mask on sync) instead of 33 small ones: a single
    dma_start is striped across all 16 SDMA engines, so descriptor-gen
    serialization (~0.6us each) was the real startup cost.
  - Normalization is per head-PAIR: the denominator reciprocal is computed
    by DVE directly from the AV PSUM row (no den staging copy), the two
    reciprocal rows of a pair live at partitions 64r/64r+1 and a single
    [2,128]-stationary PE matmul broadcasts both to the pair's 128 output
    partitions, and one [128,512] DVE multiply normalizes the whole pair.
    The broadcast PSUM tile borrows the ps_proj ring so the pav ring never
    deadlocks, and part2 (cast+broadcast+mul) is emitted one matmul group
    late so the PE queue never waits on the DVE reciprocal chain.
  - The output projection is interleaved with the last pair's norm:
    n-tiles 0..3 (query half 0) run between norm(3,qh0) and norm(3,qh1),
    n-tiles 4..7 after.

Per-core pipeline (x^T and packed weights are prepared on the host):
  1. Per head pair t (4 pairs): k^T/q^T column projections (K=co tiles),
     then S^T = k^T.T @ q^T per key m-tile, the pair alternating PE row
     groups 0/64 so its two matmuls overlap -> exp on ACT -> E.
  2. v = x @ w_v -> [m, 8 heads, d + ones column], mask folded in.
  3. Per pair and q-half: out^T (unnormalized) + denominator via the ones
     column -> o_un; fast reciprocal from PSUM, pair broadcast, normalize.
  4. y_partial = o^T.T @ w_out (K=4 e-tiles), fp16, DMA out.
"""

import os

import numpy as np

import concourse.bacc as bacc
import concourse.mybir as mybir
import concourse.tile as tile
from concourse.bass_utils import run_bass_kernel_spmd

F32 = mybir.dt.float32
F32R = mybir.dt.float32r
F16 = mybir.dt.float16

B, N, C = 4, 1024, 1024
H, D = 16, 64
P = 128
CO = C // P       # 8 contraction tiles
MO = N // P       # 8 key m-tiles
NO = N // P       # 8 output row tiles
HL = 8            # heads per core
T = HL // 2       # 4 head pairs per core
EO = T            # 4 e-tiles (one per pair) for the output projection
NH = N // 2       # 512-column matmul streams (PSUM bank)
ATT_SCALE = D ** -0.5
EXP_BIAS = float(-12.0 * np.log(2.0))  # keep out^T in fp16 range
N_CORES = 8


def build_nc():
    nc = bacc.Bacc()
    xbT = nc.declare_dram_parameter("xbT", [C, N], F16, isOutput=False)
    maskb = nc.declare_dram_parameter("maskb", [N], F32, isOutput=False)
    wq_pk = nc.declare_dram_parameter("wq_pk", [T, P, CO, P], F16,
                                      isOutput=False)
    wk_pk = nc.declare_dram_parameter("wk_pk", [T, P, CO, P], F16,
                                      isOutput=False)
    wv_pk = nc.declare_dram_parameter("wv_pk", [P, CO, HL * D], F16,
                                      isOutput=False)
    wo_pk = nc.declare_dram_parameter("wo_pk", [P, EO, C], F16, isOutput=False)
    ones2b = nc.declare_dram_parameter("ones2b", [P, P], F16, isOutput=False)
    y = nc.declare_dram_parameter("y", [N, C], F16, isOutput=True)
    debug = os.environ.get("KERNEL_DEBUG_DUMP") == "1"
    if debug:
        dbg_rcp = nc.declare_dram_parameter("dbg_rcp", [P, N], F32,
                                            isOutput=True)
        dbg_osb = nc.declare_dram_parameter("dbg_osb", [P, EO, N], F16,
                                            isOutput=True)
        dbg_kT = nc.declare_dram_parameter("dbg_kT", [P, T, N], F16,
                                           isOutput=True)
        dbg_vsb = nc.declare_dram_parameter("dbg_vsb", [P, MO, HL, P], F16,
                                            isOutput=True)

    xbT_t = xbT.rearrange("(co p) m -> p co m", p=P)
    y_t = y.rearrange("(no p) c -> p no c", p=P)

    with tile.TileContext(nc) as tc:
        with tc.tile_pool(name="consts", bufs=1) as consts, \
             tc.tile_pool(name="persist", bufs=1) as persist:
            # ---- constants ----
            # zero scratch for PE warm-up matmuls (first DVE op emitted so
            # the dummies can start as early as possible)
            scratch = consts.tile([P, NH], F16)
            nc.vector.memset(scratch[:], 0.0)
            onesH = consts.tile([P, HL], F16)
            nc.vector.memset(onesH[:], 1.0)
            # pair-broadcast stationary: row 64r+32j has ones in cols
            # 64j..64j+63, all other rows zero.  DMA'd from DRAM: engine
            # writes must be 32-partition aligned.
            ones2 = consts.tile([P, P], F16)
            mask_sb = consts.tile([P, MO], F32)
            ebias = consts.tile([P, 1], F32)
            nc.vector.memset(ebias[:], EXP_BIAS)

            # ---- persistent tensors ----
            qT = persist.tile([P, T, N], F16)            # q^T: [e, n]
            kT = persist.tile([P, T, N], F16)            # k^T: [e, m]
            v_sb = persist.tile([P, MO, HL, P], F16)     # v + ones col + pad
            o_sb = persist.tile([P, EO, N], F16)         # o^T: [e, n]
            wo = persist.tile([P, EO, C], F16)           # w_out staged late
            # den/rcp rows for head h=2t+j live at partition 64(t%2)+32j:
            # engine ops can only address partition bases 0/32/64/96, and
            # the custom-DVE reciprocal only works at base 0 (full tile).
            den_sb = persist.tile([97, N], F32)
            nc.vector.memset(den_sb[:], 1.0)             # keep rcp defined
            rcp32 = persist.tile([97, N], F32)
            rcp16 = persist.tile([P, N], F16)
            nc.vector.memset(rcp16[:], 0.0)              # unwritten rows: no
                                                         # NaN under the bcast

            with tc.tile_pool(name="xT_pool", bufs=1) as xT_pool, \
                 tc.tile_pool(name="w_pool", bufs=4) as w_pool, \
                 tc.tile_pool(name="wv_pool", bufs=1) as wv_pool, \
                 tc.tile_pool(name="E_pool", bufs=3) as E_pool, \
                 tc.tile_pool(name="ou_pool", bufs=2) as ou_pool, \
                 tc.tile_pool(name="ysb_pool", bufs=4) as ysb_pool, \
                 tc.tile_pool(name="ps_proj", bufs=2, space="PSUM") as ps_proj, \
                 tc.tile_pool(name="ps_s", bufs=2, space="PSUM") as ps_s, \
                 tc.tile_pool(name="ps_av", bufs=2, space="PSUM") as ps_av:
                xT = xT_pool.tile([P, CO, N], F16)       # x^T: [c, m]

                # ---- startup DMA: few LARGE descriptors (each dma_start is
                # striped across all 16 SDMA engines; it's the per-descriptor
                # generation cost ~0.6us that serializes on the queue).
                # sync: wk0, x(h0), wq0, mask; scalar: x(h1) pieces.
                wk0 = w_pool.tile([P, CO, P], F16, tag="wqk")
                wq0 = w_pool.tile([P, CO, P], F16, tag="wqk")
                nc.sync.dma_start(wk0[:], wk_pk[0])
                nc.scalar.dma_start(xT[:, 0:2, 0:NH], xbT_t[:, 0:2, 0:NH])
                nc.sync.dma_start(xT[:, 2:4, 0:NH], xbT_t[:, 2:4, 0:NH])
                nc.scalar.dma_start(xT[:, 4:6, 0:NH], xbT_t[:, 4:6, 0:NH])
                nc.sync.dma_start(xT[:, 6:CO, 0:NH], xbT_t[:, 6:CO, 0:NH])
                nc.sync.dma_start(wq0[:], wq_pk[0])
                nc.scalar.dma_start(xT[:, 0:4, NH:N], xbT_t[:, 0:4, NH:N])
                nc.sync.dma_start(xT[:, 4:CO, NH:N], xbT_t[:, 4:CO, NH:N])

                # PE warm-up: dummy matmuls on zero scratch while the first
                # x/w descriptors land, so the p-state ramp (first ~3us of
                # matmuls run ~2.8x slow) is paid before real work arrives
                for w in range(6):
                    pw = ps_av.tile([P, NH], F32, tag="ps_av",
                                    name=f"warm{w}")
                    nc.tensor.matmul(pw[:], scratch[:, 0:P], scratch[:],
                                     start=True, stop=True)

                def kq_half(t, half, wk, wq):
                    pk = ps_proj.tile([P, NH], F32, tag="pp",
                                      name=f"pk{t}_{half}")
                    for co in range(CO):
                        nc.tensor.matmul(
                            pk[:], wk[:, co, :],
                            xT[:, co, half * NH:(half + 1) * NH],
                            start=(co == 0), stop=(co == CO - 1))
                    nc.vector.tensor_copy(
                        kT[:, t, half * NH:(half + 1) * NH], pk[:])
                    pq = ps_proj.tile([P, NH], F32, tag="pp",
                                      name=f"pq{t}_{half}")
                    for co in range(CO):
                        nc.tensor.matmul(
                            pq[:], wq[:, co, :],
                            xT[:, co, half * NH:(half + 1) * NH],
                            start=(co == 0), stop=(co == CO - 1))
                    nc.vector.tensor_copy(
                        qT[:, t, half * NH:(half + 1) * NH], pq[:])

                def kq_proj(t, wk=None, wq=None):
                    if wk is None:
                        wk = w_pool.tile([P, CO, P], F16, tag="wqk",
                                         name=f"wk{t}")
                        nc.sync.dma_start(wk[:], wk_pk[t])
                    if wq is None:
                        wq = w_pool.tile([P, CO, P], F16, tag="wqk",
                                         name=f"wq{t}")
                        nc.sync.dma_start(wq[:], wq_pk[t])
                    kq_half(t, 0, wk, wq)
                    kq_half(t, 1, wk, wq)

                def s_exp_tiles(t, E_t, mo_list, qh_list):
                    # S^T + exp: one [128, 1024] psum per (mo, q-half) holds
                    # both heads of the pair (the back-to-back matmuls
                    # alternate PE row groups 0/64 and overlap on the array);
                    # one strided EXP covers both heads
                    for mo in mo_list:
                        for qh in qh_list:
                            pss = ps_s.tile([P, 2 * NH], F32, tag="ps_s",
                                            name=f"pss{t}_{mo}_{qh}")
                            for j in range(2):
                                pb = 64 * j
                                nc.tensor.matmul(
                                    pss[:, j * NH:(j + 1) * NH],
                                    kT[pb:pb + 64, t, mo * P:(mo + 1) * P],
                                    qT[pb:pb + 64, t, qh * NH:(qh + 1) * NH],
                                    start=True, stop=True)
                            nc.scalar.activation(
                                E_t[:, mo, :, qh * NH:(qh + 1) * NH],
                                pss[:].rearrange("p (j n) -> p j n", j=2),
                                mybir.ActivationFunctionType.Exp,
                                bias=ebias[:], scale=ATT_SCALE)

                def s_exp(t):
                    E_t = E_pool.tile([P, MO, 2, N], F16, tag="E",
                                      name=f"E{t}")
                    s_exp_tiles(t, E_t, range(MO), range(2))
                    return E_t

                def v_proj():
                    wv = wv_pool.tile([P, CO, HL * D], F16, tag="wv")
                    nc.sync.dma_start(wv[:], wv_pk[:])
                    for mo in range(MO):
                        pv = ps_proj.tile([P, NH], F32, tag="pp",
                                          name=f"pv{mo}")
                        for co in range(CO):
                            nc.tensor.matmul(
                                pv[:], xT[:, co, mo * P:(mo + 1) * P],
                                wv[:, co, :],
                                start=(co == 0), stop=(co == CO - 1))
                        nc.vector.tensor_scalar_mul(
                            v_sb[:, mo, :, 0:D],
                            pv[:].rearrange("p (h d) -> p h d", d=D),
                            mask_sb[:, mo:mo + 1])
                    for mo in range(MO):
                        nc.vector.tensor_scalar_mul(
                            v_sb[:, mo, :, D], onesH[:], mask_sb[:, mo:mo + 1])

                o_uns = {}

                def norm_part2(t, qh):
                    # cast the pair's two reciprocal rows (64r, 64r+32; the
                    # 31 rows between are zero so one 33-row cast is safe),
                    # broadcast them to the pair's 128 output partitions with
                    # one PE matmul (K=64 stationary, zeros except the two
                    # one-rows), then one [128,512] DVE multiply normalizes
                    # the whole pair.  Emitted >=1 matmul group after the
                    # reciprocals so the in-order PE queue never waits on the
                    # DVE chain.
                    r = t % 2
                    sl = slice(qh * NH, (qh + 1) * NH)
                    nc.vector.reciprocal_approx_fast(rcp32[:, sl],
                                                     den_sb[:, sl])
                    nc.vector.tensor_copy(rcp16[64 * r:64 * r + 33, sl],
                                          rcp32[64 * r:64 * r + 33, sl])
                    pbc = ps_proj.tile([P, NH], F32, tag="pp",
                                       name=f"pbc{t}_{qh}")
                    nc.tensor.matmul(
                        pbc[:], ones2[64 * r:64 * r + 64, :],
                        rcp16[64 * r:64 * r + 64, sl],
                        start=True, stop=True, tile_position=(64 * r, 0))
                    o_un = o_uns[t] if qh == 0 else o_uns.pop(t)
                    nc.vector.tensor_mul(o_sb[:, t, sl], o_un[:, sl], pbc[:])

                def av(t, E_t, mid=None):
                    # unnormalized out^T + denominator via the ones column;
                    # `mid` (if set) is emitted between the two query halves
                    # and takes over part2(t, 0) emission
                    r = t % 2
                    o_un = ou_pool.tile([P, N], F16, tag="ou", name=f"ou{t}")
                    o_uns[t] = o_un
                    for qh in range(2):
                        sl = slice(qh * NH, (qh + 1) * NH)
                        for j in range(2):
                            h = 2 * t + j
                            pav = ps_av.tile([P, NH], F32, tag="ps_av",
                                             name=f"pav{h}_{qh}")
                            for mo in range(MO):
                                nc.tensor.matmul(
                                    pav[:], v_sb[:, mo, h, :],
                                    E_t[:, mo, j, sl],
                                    start=(mo == 0), stop=(mo == MO - 1))
                            nc.vector.tensor_copy(
                                o_un[64 * j:64 * j + 64, sl], pav[0:D, :])
                            nc.vector.tensor_copy(
                                den_sb[64 * r + 32 * j:64 * r + 32 * j + 1,
                                       sl],
                                pav[D:D + 1, :])
                        if qh == 0 and mid is not None:
                            mid()
                        if qh == 1 and mid is None:
                            norm_part2(t, 0)

                def out_proj(no_list, last=False):
                    # one merged y descriptor per n-tile ([128, 1024]); the
                    # very last tile is split across both hwdge queues so the
                    # final transfer tail is halved
                    for no in no_list:
                        ysb = ysb_pool.tile([P, N], F16, tag="ysb",
                                            name=f"ysb{no}")
                        for ch in range(2):
                            pool = ps_proj if (no * 2 + ch) % 2 == 0 else ps_av
                            py = pool.tile([P, NH], F32,
                                           tag="pp" if pool is ps_proj
                                           else "ps_av",
                                           name=f"py{no}_{ch}")
                            for eo in range(EO):
                                nc.tensor.matmul(
                                    py[:], o_sb[:, eo, no * P:(no + 1) * P],
                                    wo[:, eo, ch * NH:(ch + 1) * NH],
                                    start=(eo == 0), stop=(eo == EO - 1))
                            if last and no == no_list[-1]:
                                if ch == 0:
                                    nc.scalar.copy(
                                        ysb[:, 0:NH], py[:])
                                    nc.sync.dma_start(
                                        y_t[:, no, 0:NH], ysb[:, 0:NH])
                                else:
                                    nc.vector.tensor_copy(
                                        ysb[:, NH:N], py[:])
                                    nc.sync.dma_start(
                                        y_t[:, no, NH:NH + 256],
                                        ysb[:, NH:NH + 256])
                                    nc.scalar.dma_start(
                                        y_t[:, no, NH + 256:N],
                                        ysb[:, NH + 256:N])
                            else:
                                # ch0 on ACT, ch1 on DVE so neither engine
                                # paces the out-proj matmul stream
                                if ch == 0:
                                    nc.scalar.copy(ysb[:, 0:NH], py[:])
                                else:
                                    nc.vector.tensor_copy(
                                        ysb[:, NH:N], py[:])
                        if not (last and no == no_list[-1]):
                            q = nc.sync if no % 2 == 0 else nc.scalar
                            q.dma_start(y_t[:, no, :], ysb[:])

                # skewed pipeline: attention of pair t overlaps k/q
                # projections of pair t+1 and S/exp of pair t+1.
                # pair-0 ramp: project the first query half, then run its
                # first S tiles + exps while x's second half is still landing
                kq_half(0, 0, wk0, wq0)
                E0 = E_pool.tile([P, MO, 2, N], F16, tag="E", name="E0")
                s_exp_tiles(0, E0, range(4), [0])
                kq_half(0, 1, wk0, wq0)
                s_exp_tiles(0, E0, range(4), [1])
                s_exp_tiles(0, E0, range(4, MO), range(2))
                Es = {0: E0}
                kq_proj(1)
                # non-critical input DMAs + the AV weight-pad memset go after
                # the pair-1 weight descriptors so they can't delay the ramp
                nc.sync.dma_start(mask_sb[:],
                                  maskb.rearrange("(o p) -> p o", p=P))
                nc.sync.dma_start(ones2[:], ones2b[:])
                nc.vector.memset(v_sb[:, :, :, D + 1:], 0.0)
                Es[1] = s_exp(1)
                kq_proj(2)
                v_proj()
                for t in range(T):
                    if t >= 1:
                        norm_part2(t - 1, 1)
                    if t + 2 < T:
                        E_n = E_pool.tile([P, MO, 2, N], F16, tag="E",
                                          name=f"E{t + 2}")
                        if t + 2 == T - 1:
                            # the last pair's exps run qh-major so its qh0
                            # AV/norm (and the first out-proj tiles) can
                            # overlap the qh1 exp stream
                            s_exp_tiles(t + 2, E_n, range(MO), [0])
                            s_exp_tiles(t + 2, E_n, range(MO), [1])
                        else:
                            s_exp_tiles(t + 2, E_n, range(MO), range(2))
                        Es[t + 2] = E_n
                    if t + 3 < T:
                        kq_proj(t + 3)
                    if t == 1:  # stage w_out late, off the critical DMA path
                        nc.sync.dma_start(wo[:], wo_pk[:])
                    if t == T - 1:
                        # last pair: norm qh0 + the first out-proj n-tiles
                        # run between the query halves, inside the qh1 exp
                        # window; the rest follows after qh1's norm
                        av(t, Es.pop(t),
                           mid=lambda: (norm_part2(3, 0),
                                        out_proj([0, 1, 2])))
                    else:
                        av(t, Es.pop(t))

                # ---- remaining output projection (host adds bias)
                out_proj([3])
                norm_part2(3, 1)
                out_proj([4, 5, 6, 7], last=True)

                if debug:
                    nc.sync.dma_start(dbg_rcp[0:97], rcp32[:])
                    nc.sync.dma_start(dbg_osb[:], o_sb[:])
                    nc.sync.dma_start(dbg_kT[:], kT[:])
                    nc.sync.dma_start(dbg_vsb[:], v_sb[:])

    nc.finalize()
    return nc


_NC_CACHE = None


def _get_nc():
    global _NC_CACHE
    if _NC_CACHE is None:
        _NC_CACHE = build_nc()
    return _NC_CACHE


def _make_in_maps(x, mask, w_qkv, w_out, b_out):
    x = np.asarray(x, dtype=np.float32)
    mask_f = np.asarray(mask).astype(np.float32)
    wqkv_h = np.asarray(w_qkv).astype(np.float16)
    wout_h = np.asarray(w_out).astype(np.float16)
    # w_qkv [C, 3HD]: q cols 0:C, k cols C:2C, v cols 2C:3C; head h at h*D
    wq4 = wqkv_h.reshape(CO, P, 3 * H * D)
    in_maps = []
    for i in range(N_CORES):
        b, hh = i // 2, i % 2
        e0 = hh * HL * D  # first e-col of this core's head block
        xbT = np.ascontiguousarray(x[b].T.astype(np.float16))
        wq_pk = np.ascontiguousarray(
            wq4[:, :, e0:e0 + HL * D].reshape(CO, P, T, P)
            .transpose(2, 1, 0, 3))
        wk_pk = np.ascontiguousarray(
            wq4[:, :, C + e0:C + e0 + HL * D].reshape(CO, P, T, P)
            .transpose(2, 1, 0, 3))
        wv_pk = np.ascontiguousarray(
            wq4[:, :, 2 * C + e0:2 * C + e0 + HL * D].transpose(1, 0, 2))
        wo_pk = np.ascontiguousarray(
            wout_h[e0:e0 + HL * D, :].reshape(EO, P, C).transpose(1, 0, 2))
        ones2b = np.zeros((P, P), np.float16)
        for r in range(2):
            ones2b[64 * r, 0:64] = 1.0
            ones2b[64 * r + 32, 64:128] = 1.0
        in_maps.append({"xbT": xbT, "maskb": mask_f[b], "wq_pk": wq_pk,
                        "wk_pk": wk_pk, "wv_pk": wv_pk, "wo_pk": wo_pk,
                        "ones2b": ones2b})
    return in_maps


def run_kernel(x, mask, w_qkv, w_out, b_out, trace=False):
    """Run on 8 cores; returns (full output [B,N,C], BassKernelResults)."""
    nc = _get_nc()
    in_maps = _make_in_maps(x, mask, w_qkv, w_out, b_out)
    res = run_bass_kernel_spmd(nc, in_maps, core_ids=list(range(N_CORES)),
                               trace=trace)
    bias = np.asarray(b_out, dtype=np.float32)
    out = np.empty((B, N, C), dtype=np.float32)
    for b in range(B):
        out[b] = (res.results[2 * b]["y"].astype(np.float32)
                  + res.results[2 * b + 1]["y"].astype(np.float32) + bias)
    return out, res


def kernel(x, mask, w_qkv, w_out, b_out):
    os.environ.setdefault("BASS_NEVER_TRACE", "1")
    out, _ = run_kernel(x, mask, w_qkv, w_out, b_out, trace=False)
    return out


# revision 44
# speedup vs baseline: 1.0227x; 1.0077x over previous
"""Multi-head attention (B=4, N=1024, C=1024, H=16, D=64) on 8 Trainium2 cores.

Sharding: batch x head-half tensor parallel, no collectives. Core i handles
batch b = i//2 and heads (i%2)*8..+8 for ALL 1024 queries of that batch: it
projects q/k/v for its 8 heads only (no duplicated k/v work between the two
cores of a batch), runs attention, and computes the PARTIAL output projection
y_i = o_i @ w_out[rows of its 512 e-dims]. The host sums each batch's two
partials and adds the bias -- the output projection is linear in the head
dimension, so the pair-sum equals the full projection.

Matmuls run in fp16 (1 PE column/cycle, weight loads hidden under streams).
Accumulation is fp32 in PSUM. exp is computed as exp(s/8 - 12*ln2) so
unnormalized attention outputs stay in fp16 range; the 2^-12 factor cancels
in the softmax normalization. The softmax denominator rides along as a
ones-column in v (key mask folded into both); v tiles are padded to 128
weight columns (65..127 zero) so AV matmuls get FWL.

v2 schedule changes vs v1:
  - Input DMA is 7 large descriptors (x in 4 quarters split across the two
    hwdge queues, wk0/wq0/mask on sync) instead of 33 small ones: a single
    dma_start is striped across all 16 SDMA engines, so descriptor-gen
    serialization (~0.6us each) was the real startup cost.
  - Normalization is per head-PAIR: the denominator reciprocal is computed
    by DVE directly from the AV PSUM row (no den staging copy), the two
    reciprocal rows of a pair live at partitions 64r/64r+1 and a single
    [2,128]-stationary PE matmul broadcasts both to the pair's 128 output
    partitions, and one [128,512] DVE multiply normalizes the whole pair.
    The broadcast PSUM tile borrows the ps_proj ring so the pav ring never
    deadlocks, and part2 (cast+broadcast+mul) is emitted one matmul group
    late so the PE queue never waits on the DVE reciprocal chain.
  - The output projection is interleaved with the last pair's norm:
    n-tiles 0..3 (query half 0) run between norm(3,qh0) and norm(3,qh1),
    n-tiles 4..7 after.

Per-core pipeline (x^T and packed weights are prepared on the host):
  1. Per head pair t (4 pairs): k^T/q^T column projections (K=co tiles),
     then S^T = k^T.T @ q^T per key m-tile, the pair alternating PE row
     groups 0/64 so its two matmuls overlap -> exp on ACT -> E.
  2. v = x @ w_v -> [m, 8 heads, d + ones column], mask folded in.
  3. Per pair and q-half: out^T (unnormalized) + denominator via the ones
     column -> o_un; fast reciprocal from PSUM, pair broadcast, normalize.
  4. y_partial = o^T.T @ w_out (K=4 e-tiles), fp16, DMA out.
"""

import os

import numpy as np

import concourse.bacc as bacc
import concourse.mybir as mybir
import concourse.tile as tile
from concourse.bass_utils import run_bass_kernel_spmd

F32 = mybir.dt.float32
F32R = mybir.dt.float32r
F16 = mybir.dt.float16

B, N, C = 4, 1024, 1024
H, D = 16, 64
P = 128
CO = C // P       # 8 contraction tiles
MO = N // P       # 8 key m-tiles
NO = N // P       # 8 output row tiles
HL = 8            # heads per core
T = HL // 2       # 4 head pairs per core
EO = T            # 4 e-tiles (one per pair) for the output projection
NH = N // 2       # 512-column matmul streams (PSUM bank)
ATT_SCALE = D ** -0.5
EXP_BIAS = float(-12.0 * np.log(2.0))  # keep out^T in fp16 range
N_CORES = 8


def build_nc():
    nc = bacc.Bacc()
    xbT = nc.declare_dram_parameter("xbT", [C, N], F16, isOutput=False)
    maskb = nc.declare_dram_parameter("maskb", [N], F32, isOutput=False)
    wq_pk = nc.declare_dram_parameter("wq_pk", [T, P, CO, P], F16,
                                      isOutput=False)
    wk_pk = nc.declare_dram_parameter("wk_pk", [T, P, CO, P], F16,
                                      isOutput=False)
    wv_pk = nc.declare_dram_parameter("wv_pk", [P, CO, HL * D], F16,
                                      isOutput=False)
    wo_pk = nc.declare_dram_parameter("wo_pk", [P, EO, C], F16, isOutput=False)
    ones2b = nc.declare_dram_parameter("ones2b", [P, P], F16, isOutput=False)
    y = nc.declare_dram_parameter("y", [N, C], F16, isOutput=True)
    debug = os.environ.get("KERNEL_DEBUG_DUMP") == "1"
    if debug:
        dbg_rcp = nc.declare_dram_parameter("dbg_rcp", [P, N], F32,
                                            isOutput=True)
        dbg_osb = nc.declare_dram_parameter("dbg_osb", [P, EO, N], F16,
                                            isOutput=True)
        dbg_kT = nc.declare_dram_parameter("dbg_kT", [P, T, N], F16,
                                           isOutput=True)
        dbg_vsb = nc.declare_dram_parameter("dbg_vsb", [P, MO, HL, P], F16,
                                            isOutput=True)

    xbT_t = xbT.rearrange("(co p) m -> p co m", p=P)
    y_t = y.rearrange("(no p) c -> p no c", p=P)

    with tile.TileContext(nc) as tc:
        with tc.tile_pool(name="consts", bufs=1) as consts, \
             tc.tile_pool(name="persist", bufs=1) as persist:
            # ---- constants ----
            # zero scratch for PE warm-up matmuls (first DVE op emitted so
            # the dummies can start as early as possible)
            scratch = consts.tile([P, NH], F16)
            nc.vector.memset(scratch[:], 0.0)
            onesH = consts.tile([P, HL], F16)
            nc.vector.memset(onesH[:], 1.0)
            # pair-broadcast stationary: row 64r+32j has ones in cols
            # 64j..64j+63, all other rows zero.  DMA'd from DRAM: engine
            # writes must be 32-partition aligned.
            ones2 = consts.tile([P, P], F16)
            mask_sb = consts.tile([P, MO], F32)
            ebias = consts.tile([P, 1], F32)
            nc.vector.memset(ebias[:], EXP_BIAS)

            # ---- persistent tensors ----
            qT = persist.tile([P, T, N], F16)            # q^T: [e, n]
            kT = persist.tile([P, T, N], F16)            # k^T: [e, m]
            v_sb = persist.tile([P, MO, HL, P], F16)     # v + ones col + pad
            o_sb = persist.tile([P, EO, N], F16)         # o^T: [e, n]
            wo = persist.tile([P, EO, C], F16)           # w_out staged late
            # den/rcp rows for head h=2t+j live at partition 64(t%2)+32j:
            # engine ops can only address partition bases 0/32/64/96, and
            # the custom-DVE reciprocal only works at base 0 (full tile).
            den_sb = persist.tile([97, N], F32)
            nc.vector.memset(den_sb[:], 1.0)             # keep rcp defined
            rcp32 = persist.tile([97, N], F32)
            rcp16 = persist.tile([P, N], F16)
            nc.vector.memset(rcp16[:], 0.0)              # unwritten rows: no
                                                         # NaN under the bcast

            with tc.tile_pool(name="xT_pool", bufs=1) as xT_pool, \
                 tc.tile_pool(name="w_pool", bufs=4) as w_pool, \
                 tc.tile_pool(name="wv_pool", bufs=1) as wv_pool, \
                 tc.tile_pool(name="E_pool", bufs=3) as E_pool, \
                 tc.tile_pool(name="ou_pool", bufs=2) as ou_pool, \
                 tc.tile_pool(name="ysb_pool", bufs=4) as ysb_pool, \
                 tc.tile_pool(name="ps_proj", bufs=2, space="PSUM") as ps_proj, \
                 tc.tile_pool(name="ps_s", bufs=2, space="PSUM") as ps_s, \
                 tc.tile_pool(name="ps_av", bufs=2, space="PSUM") as ps_av:
                xT = xT_pool.tile([P, CO, N], F16)       # x^T: [c, m]

                # ---- startup DMA: few LARGE descriptors (each dma_start is
                # striped across all 16 SDMA engines; it's the per-descriptor
                # generation cost ~0.6us that serializes on the queue).
                # sync: wk0, x(h0), wq0, mask; scalar: x(h1) pieces.
                wk0 = w_pool.tile([P, CO, P], F16, tag="wqk")
                wq0 = w_pool.tile([P, CO, P], F16, tag="wqk")
                nc.sync.dma_start(wk0[:], wk_pk[0])
                nc.scalar.dma_start(xT[:, 0:2, 0:NH], xbT_t[:, 0:2, 0:NH])
                nc.sync.dma_start(xT[:, 2:4, 0:NH], xbT_t[:, 2:4, 0:NH])
                nc.scalar.dma_start(xT[:, 4:6, 0:NH], xbT_t[:, 4:6, 0:NH])
                nc.sync.dma_start(xT[:, 6:CO, 0:NH], xbT_t[:, 6:CO, 0:NH])
                nc.sync.dma_start(wq0[:], wq_pk[0])
                nc.scalar.dma_start(xT[:, 0:4, NH:N], xbT_t[:, 0:4, NH:N])
                nc.sync.dma_start(xT[:, 4:CO, NH:N], xbT_t[:, 4:CO, NH:N])


                def kq_half(t, half, wk, wq):
                    pk = ps_proj.tile([P, NH], F32, tag="pp",
                                      name=f"pk{t}_{half}")
                    for co in range(CO):
                        nc.tensor.matmul(
                            pk[:], wk[:, co, :],
                            xT[:, co, half * NH:(half + 1) * NH],
                            start=(co == 0), stop=(co == CO - 1))
                    nc.vector.tensor_copy(
                        kT[:, t, half * NH:(half + 1) * NH], pk[:])
                    pq = ps_proj.tile([P, NH], F32, tag="pp",
                                      name=f"pq{t}_{half}")
                    for co in range(CO):
                        nc.tensor.matmul(
                            pq[:], wq[:, co, :],
                            xT[:, co, half * NH:(half + 1) * NH],
                            start=(co == 0), stop=(co == CO - 1))
                    nc.vector.tensor_copy(
                        qT[:, t, half * NH:(half + 1) * NH], pq[:])

                def kq_proj(t, wk=None, wq=None):
                    if wk is None:
                        wk = w_pool.tile([P, CO, P], F16, tag="wqk",
                                         name=f"wk{t}")
                        nc.sync.dma_start(wk[:], wk_pk[t])
                    if wq is None:
                        wq = w_pool.tile([P, CO, P], F16, tag="wqk",
                                         name=f"wq{t}")
                        nc.sync.dma_start(wq[:], wq_pk[t])
                    kq_half(t, 0, wk, wq)
                    kq_half(t, 1, wk, wq)

                def s_exp_tiles(t, E_t, mo_list, qh_list):
                    # S^T + exp: one [128, 1024] psum per (mo, q-half) holds
                    # both heads of the pair (the back-to-back matmuls
                    # alternate PE row groups 0/64 and overlap on the array);
                    # one strided EXP covers both heads
                    for mo in mo_list:
                        for qh in qh_list:
                            pss = ps_s.tile([P, 2 * NH], F32, tag="ps_s",
                                            name=f"pss{t}_{mo}_{qh}")
                            for j in range(2):
                                pb = 64 * j
                                nc.tensor.matmul(
                                    pss[:, j * NH:(j + 1) * NH],
                                    kT[pb:pb + 64, t, mo * P:(mo + 1) * P],
                                    qT[pb:pb + 64, t, qh * NH:(qh + 1) * NH],
                                    start=True, stop=True)
                            nc.scalar.activation(
                                E_t[:, mo, :, qh * NH:(qh + 1) * NH],
                                pss[:].rearrange("p (j n) -> p j n", j=2),
                                mybir.ActivationFunctionType.Exp,
                                bias=ebias[:], scale=ATT_SCALE)

                def s_exp(t):
                    E_t = E_pool.tile([P, MO, 2, N], F16, tag="E",
                                      name=f"E{t}")
                    s_exp_tiles(t, E_t, range(MO), range(2))
                    return E_t

                def v_proj():
                    wv = wv_pool.tile([P, CO, HL * D], F16, tag="wv")
                    nc.sync.dma_start(wv[:], wv_pk[:])
                    for mo in range(MO):
                        pv = ps_proj.tile([P, NH], F32, tag="pp",
                                          name=f"pv{mo}")
                        for co in range(CO):
                            nc.tensor.matmul(
                                pv[:], xT[:, co, mo * P:(mo + 1) * P],
                                wv[:, co, :],
                                start=(co == 0), stop=(co == CO - 1))
                        nc.vector.tensor_scalar_mul(
                            v_sb[:, mo, :, 0:D],
                            pv[:].rearrange("p (h d) -> p h d", d=D),
                            mask_sb[:, mo:mo + 1])
                    for mo in range(MO):
                        nc.vector.tensor_scalar_mul(
                            v_sb[:, mo, :, D], onesH[:], mask_sb[:, mo:mo + 1])

                o_uns = {}

                def norm_part2(t, qh):
                    # cast the pair's two reciprocal rows (64r, 64r+32; the
                    # 31 rows between are zero so one 33-row cast is safe),
                    # broadcast them to the pair's 128 output partitions with
                    # one PE matmul (K=64 stationary, zeros except the two
                    # one-rows), then one [128,512] DVE multiply normalizes
                    # the whole pair.  Emitted >=1 matmul group after the
                    # reciprocals so the in-order PE queue never waits on the
                    # DVE chain.
                    r = t % 2
                    sl = slice(qh * NH, (qh + 1) * NH)
                    nc.vector.reciprocal_approx_fast(rcp32[:, sl],
                                                     den_sb[:, sl])
                    nc.vector.tensor_copy(rcp16[64 * r:64 * r + 33, sl],
                                          rcp32[64 * r:64 * r + 33, sl])
                    pbc = ps_proj.tile([P, NH], F32, tag="pp",
                                       name=f"pbc{t}_{qh}")
                    nc.tensor.matmul(
                        pbc[:], ones2[64 * r:64 * r + 64, :],
                        rcp16[64 * r:64 * r + 64, sl],
                        start=True, stop=True, tile_position=(64 * r, 0))
                    o_un = o_uns[t] if qh == 0 else o_uns.pop(t)
                    nc.vector.tensor_mul(o_sb[:, t, sl], o_un[:, sl], pbc[:])

                def av(t, E_t, mid=None):
                    # unnormalized out^T + denominator via the ones column;
                    # `mid` (if set) is emitted between the two query halves
                    # and takes over part2(t, 0) emission
                    r = t % 2
                    o_un = ou_pool.tile([P, N], F16, tag="ou", name=f"ou{t}")
                    o_uns[t] = o_un
                    for qh in range(2):
                        sl = slice(qh * NH, (qh + 1) * NH)
                        for j in range(2):
                            h = 2 * t + j
                            pav = ps_av.tile([P, NH], F32, tag="ps_av",
                                             name=f"pav{h}_{qh}")
                            for mo in range(MO):
                                nc.tensor.matmul(
                                    pav[:], v_sb[:, mo, h, :],
                                    E_t[:, mo, j, sl],
                                    start=(mo == 0), stop=(mo == MO - 1))
                            nc.vector.tensor_copy(
                                o_un[64 * j:64 * j + 64, sl], pav[0:D, :])
                            nc.vector.tensor_copy(
                                den_sb[64 * r + 32 * j:64 * r + 32 * j + 1,
                                       sl],
                                pav[D:D + 1, :])
                        if qh == 0 and mid is not None:
                            mid()
                        if qh == 1 and mid is None:
                            norm_part2(t, 0)

                def out_proj(no_list, last=False):
                    # one merged y descriptor per n-tile ([128, 1024]); the
                    # very last tile is split across both hwdge queues so the
                    # final transfer tail is halved
                    for no in no_list:
                        ysb = ysb_pool.tile([P, N], F16, tag="ysb",
                                            name=f"ysb{no}")
                        for ch in range(2):
                            pool = ps_proj if (no * 2 + ch) % 2 == 0 else ps_av
                            py = pool.tile([P, NH], F32,
                                           tag="pp" if pool is ps_proj
                                           else "ps_av",
                                           name=f"py{no}_{ch}")
                            for eo in range(EO):
                                nc.tensor.matmul(
                                    py[:], o_sb[:, eo, no * P:(no + 1) * P],
                                    wo[:, eo, ch * NH:(ch + 1) * NH],
                                    start=(eo == 0), stop=(eo == EO - 1))
                            if last and no == no_list[-1]:
                                if ch == 0:
                                    nc.scalar.copy(
                                        ysb[:, 0:NH], py[:])
                                    nc.sync.dma_start(
                                        y_t[:, no, 0:NH], ysb[:, 0:NH])
                                else:
                                    nc.vector.tensor_copy(
                                        ysb[:, NH:N], py[:])
                                    nc.sync.dma_start(
                                        y_t[:, no, NH:NH + 256],
                                        ysb[:, NH:NH + 256])
                                    nc.scalar.dma_start(
                                        y_t[:, no, NH + 256:N],
                                        ysb[:, NH + 256:N])
                            else:
                                # ch0 on ACT, ch1 on DVE so neither engine
                                # paces the out-proj matmul stream
                                if ch == 0:
                                    nc.scalar.copy(ysb[:, 0:NH], py[:])
                                else:
                                    nc.vector.tensor_copy(
                                        ysb[:, NH:N], py[:])
                        if not (last and no == no_list[-1]):
                            q = nc.sync if no % 2 == 0 else nc.scalar
                            q.dma_start(y_t[:, no, :], ysb[:])

                # skewed pipeline: attention of pair t overlaps k/q
                # projections of pair t+1 and S/exp of pair t+1.
                # pair-0 ramp: project the first query half, then run its
                # first S tiles + exps while x's second half is still landing
                kq_half(0, 0, wk0, wq0)
                E0 = E_pool.tile([P, MO, 2, N], F16, tag="E", name="E0")
                s_exp_tiles(0, E0, range(4), [0])
                kq_half(0, 1, wk0, wq0)
                s_exp_tiles(0, E0, range(4), [1])
                s_exp_tiles(0, E0, range(4, MO), range(2))
                Es = {0: E0}
                kq_proj(1)
                # non-critical input DMAs + the AV weight-pad memset go after
                # the pair-1 weight descriptors so they can't delay the ramp
                nc.sync.dma_start(mask_sb[:],
                                  maskb.rearrange("(o p) -> p o", p=P))
                nc.sync.dma_start(ones2[:], ones2b[:])
                nc.vector.memset(v_sb[:, :, :, D + 1:], 0.0)
                Es[1] = s_exp(1)
                kq_proj(2)
                v_proj()
                for t in range(T):
                    if t >= 1:
                        norm_part2(t - 1, 1)
                    if t + 2 < T:
                        E_n = E_pool.tile([P, MO, 2, N], F16, tag="E",
                                          name=f"E{t + 2}")
                        if t + 2 == T - 1:
                            # the last pair's exps run qh-major so its qh0
                            # AV/norm (and the first out-proj tiles) can
                            # overlap the qh1 exp stream
                            s_exp_tiles(t + 2, E_n, range(MO), [0])
                            s_exp_tiles(t + 2, E_n, range(MO), [1])
                        else:
                            s_exp_tiles(t + 2, E_n, range(MO), range(2))
                        Es[t + 2] = E_n
                    if t + 3 < T:
                        kq_proj(t + 3)
                    if t == 1:  # stage w_out late, off the critical DMA path
                        nc.sync.dma_start(wo[:], wo_pk[:])
                    if t == T - 1:
                        # last pair: norm qh0 + the first out-proj n-tiles
                        # run between the query halves, inside the qh1 exp
                        # window; the rest follows after qh1's norm
                        av(t, Es.pop(t),
                           mid=lambda: (norm_part2(3, 0),
                                        out_proj([0, 1, 2])))
                    else:
                        av(t, Es.pop(t))

                # ---- remaining output projection (host adds bias)
                out_proj([3])
                norm_part2(3, 1)
                out_proj([4, 5, 6, 7], last=True)

                if debug:
                    nc.sync.dma_start(dbg_rcp[0:97], rcp32[:])
                    nc.sync.dma_start(dbg_osb[:], o_sb[:])
                    nc.sync.dma_start(dbg_kT[:], kT[:])
                    nc.sync.dma_start(dbg_vsb[:], v_sb[:])

    nc.finalize()
    return nc


_NC_CACHE = None


def _get_nc():
    global _NC_CACHE
    if _NC_CACHE is None:
        _NC_CACHE = build_nc()
    return _NC_CACHE


def _make_in_maps(x, mask, w_qkv, w_out, b_out):
    x = np.asarray(x, dtype=np.float32)
    mask_f = np.asarray(mask).astype(np.float32)
    wqkv_h = np.asarray(w_qkv).astype(np.float16)
    wout_h = np.asarray(w_out).astype(np.float16)
    # w_qkv [C, 3HD]: q cols 0:C, k cols C:2C, v cols 2C:3C; head h at h*D
    wq4 = wqkv_h.reshape(CO, P, 3 * H * D)
    in_maps = []
    for i in range(N_CORES):
        b, hh = i // 2, i % 2
        e0 = hh * HL * D  # first e-col of this core's head block
        xbT = np.ascontiguousarray(x[b].T.astype(np.float16))
        wq_pk = np.ascontiguousarray(
            wq4[:, :, e0:e0 + HL * D].reshape(CO, P, T, P)
            .transpose(2, 1, 0, 3))
        wk_pk = np.ascontiguousarray(
            wq4[:, :, C + e0:C + e0 + HL * D].reshape(CO, P, T, P)
            .transpose(2, 1, 0, 3))
        wv_pk = np.ascontiguousarray(
            wq4[:, :, 2 * C + e0:2 * C + e0 + HL * D].transpose(1, 0, 2))
        wo_pk = np.ascontiguousarray(
            wout_h[e0:e0 + HL * D, :].reshape(EO, P, C).transpose(1, 0, 2))
        ones2b = np.zeros((P, P), np.float16)
        for r in range(2):
            ones2b[64 * r, 0:64] = 1.0
            ones2b[64 * r + 32, 64:128] = 1.0
        in_maps.append({"xbT": xbT, "maskb": mask_f[b], "wq_pk": wq_pk,
                        "wk_pk": wk_pk, "wv_pk": wv_pk, "wo_pk": wo_pk,
                        "ones2b": ones2b})
    return in_maps


def run_kernel(x, mask, w_qkv, w_out, b_out, trace=False):
    """Run on 8 cores; returns (full output [B,N,C], BassKernelResults)."""
    nc = _get_nc()
    in_maps = _make_in_maps(x, mask, w_qkv, w_out, b_out)
    res = run_bass_kernel_spmd(nc, in_maps, core_ids=list(range(N_CORES)),
                               trace=trace)
    bias = np.asarray(b_out, dtype=np.float32)
    out = np.empty((B, N, C), dtype=np.float32)
    for b in range(B):
        out[b] = (res.results[2 * b]["y"].astype(np.float32)
                  + res.results[2 * b + 1]["y"].astype(np.float32) + bias)
    return out, res


def kernel(x, mask, w_qkv, w_out, b_out):
    os.environ.setdefault("BASS_NEVER_TRACE", "1")
    out, _ = run_kernel(x, mask, w_qkv, w_out, b_out, trace=False)
    return out
